# revision 3
# baseline (speedup 1.0000x reference)
"""Trainium2 Bass kernel for a 2-layer char-LSTM (B=64, T=512, H=1024, V=256).

Strategy (8-way tensor-parallel over the 4H gate dim, one NeuronCore chip):
- Core c owns gate columns [i_c|f_c|o_c|g_c] (128 each, gate-reordered so the
  sigmoid block is contiguous) and hidden chunk c per layer; sigmoid runs via
  the tanh identity with i/f/o weight columns prescaled by 0.5 (the HW sigmoid
  table is too inaccurate).
- The embedding + layer-0 input projection fold into Wie = embed @ Wi0_c + b0
  computed in an on-device prologue; per step the x@Wi0 term is two one-hot
  matmuls accumulated straight into the gate PSUM.
- Per tick, the only serial-critical chain is
    [h1 slots] -> z1 wh0 matmuls -> cell1 -> transpose -> DRAM -> AllGather;
  layer 2 runs two steps behind (z2 at tick w computes h2(w-2)), so its
  gate matmuls, cell, projection and its own AllGather all execute inside
  layer 1's AllGather wait window.  Each layer gathers through its own
  collective; h1 uses the SP HWDGE ring and h2 the Activation ring.
- The gathered chunks load back with one strided rearrange DMA per layer;
  keep-warm dummy matmuls bridge the collective wait so the PE HAM clock
  gate stays at 8/8 (2.4 GHz).
- Direct core-to-core SBUF DMA (remote_dma / hostgen) would cut the ~10us
  per-tick collective cost to ~3us but is non-functional in this
  environment (Q7 SWDGE descriptors crash the exec unit for every routing
  id; REMOTE_DMA_HOSTGEN aborts walrus codegen).
"""
import sys

sys.path.insert(0, "/opt/trn_rl_repo")

import numpy as np
from concourse import bacc, tile, mybir
from concourse.bass_utils import run_bass_kernel_spmd

B, T, H, V, NC = 64, 512, 1024, 256, 8
KT = H // 128
HC = H // NC             # 128 hidden dims per core
GC = 4 * H // NC         # 512 gate cols per core
CH = 16                  # steps per onehot chunk

DT = mybir.dt.float32
DTR = mybir.dt.float32r
AF = mybir.ActivationFunctionType
ALU = mybir.AluOpType

N_WARM = 12  # keep-warm dummy matmuls per tick

COMM = "agc4h"  # default variant (fp16 h gathers)


def r(ap):
    return ap.bitcast(DTR)


def build_nc4(t_steps=T, reps=1, comm="agc4"):
    """wave4: double-lagged layer 2 so only [slots -> z1 -> cell1 -> agin ->
    AllGather] sits on the per-tick critical cycle; z2 (computing h2(w-2)),
    proj (logits(w-3)) and the onehot/bias/wi1 matmuls all execute inside
    the AllGather wait window.  Slot loads alternate between the SP and
    Activation HWDGE rings to avoid single-ring FIFO serialization."""
    nc = bacc.Bacc(None, target_bir_lowering=False, num_devices=NC)

    WDT = mybir.dt.float16 if comm.startswith("agc4h") else DTR
    p_embed = nc.declare_dram_parameter("embed", [V, H], DTR, isOutput=False)
    p_wi0 = nc.declare_dram_parameter("wi0", [128, KT * GC], DTR, isOutput=False)
    p_wh0 = nc.declare_dram_parameter("wh0", [128, KT * GC], WDT, isOutput=False)
    p_wi1 = nc.declare_dram_parameter("wi1", [128, KT * GC], WDT, isOutput=False)
    p_wh1 = nc.declare_dram_parameter("wh1", [128, KT * GC], WDT, isOutput=False)
    p_wproj = nc.declare_dram_parameter("wproj", [128, KT * V], WDT, isOutput=False)
    p_b0 = nc.declare_dram_parameter("b0", [1, GC], DTR, isOutput=False)
    p_b1 = nc.declare_dram_parameter("b1", [1, GC], DTR, isOutput=False)
    p_oh = nc.declare_dram_parameter(
        "onehot", [2, 128, t_steps * B], DTR, isOutput=False
    )
    p_out = nc.declare_dram_parameter("out", [B, t_steps, V], DT, isOutput=True)

    c_ident = nc.inline_tensor(np.eye(128, dtype=np.float32), name="ident")
    c_ones = nc.inline_tensor(np.ones((1, B), dtype=np.float32), name="ones")
    c_onesr = nc.inline_tensor(np.ones((1, 128), dtype=np.float32), name="onesr")
    c_zero = nc.inline_tensor(
        np.zeros((128, NC * B), dtype=np.float32), name="zeros"
    )
    c_zeroh = nc.inline_tensor(
        np.zeros((128, NC * B), dtype=np.float16), name="zerosh"
    )
    c_onesh = nc.inline_tensor(np.ones((1, B), dtype=np.float16), name="onesh")

    local = comm.endswith("_local")
    half = comm.startswith("agc4h")
    DH = mybir.dt.float16 if half else DTR

    def rh(ap):
        return ap if half else ap.bitcast(DTR)

    from contextlib import ExitStack

    with tile.TileContext(nc) as tc, ExitStack() as stack:
        wp = stack.enter_context(tc.tile_pool(name="weights", bufs=1))
        wh0_sb = wp.tile([128, KT * GC], DH, tag="wh0")
        wi1_sb = wp.tile([128, KT * GC], DH, tag="wi1")
        wh1_sb = wp.tile([128, KT * GC], DH, tag="wh1")
        wproj_sb = wp.tile([128, KT * V], DH, tag="wproj")
        wie_sb = wp.tile([128, 2 * GC], DTR, tag="wie")
        b1_sb = wp.tile([1, GC], DTR, tag="b1")
        b0_sb = wp.tile([1, GC], DTR, tag="b0")
        ident_sb = wp.tile([128, 128], DTR, tag="ident")
        ones_sb = wp.tile([1, B], DTR, tag="ones")
        onesr_sb = wp.tile([1, 128], DTR, tag="onesr")
        zero_sb = wp.tile([128, NC * B], DH, tag="zero")

        nc.sync.dma_start(wh0_sb[:], p_wh0[:])
        nc.sync.dma_start(wi1_sb[:], p_wi1[:])
        nc.sync.dma_start(wh1_sb[:], p_wh1[:])
        nc.sync.dma_start(wproj_sb[:], p_wproj[:])
        nc.sync.dma_start(b1_sb[:], p_b1[:])
        nc.sync.dma_start(b0_sb[:], p_b0[:])
        nc.gpsimd.dma_start(ident_sb[:], c_ident[:])
        nc.gpsimd.dma_start(ones_sb[:], c_ones[:])
        nc.gpsimd.dma_start(onesr_sb[:], c_onesr[:])
        if half:
            nc.gpsimd.dma_start(zero_sb[:], c_zeroh[:])
        else:
            nc.gpsimd.dma_start(zero_sb[:], c_zero[:].bitcast(DTR))

        with (
            tc.tile_pool(name="prolog", bufs=1) as pp,
            tc.tile_pool(name="prolog_ps", bufs=2, space="PSUM") as pps,
        ):
            wi0_sb = pp.tile([128, KT * GC], DTR, tag="wi0")
            em_sb = pp.tile([128, 2 * H], DTR, tag="em")
            emt_sb = pp.tile([128, KT * V], DTR, tag="emt")
            nc.sync.dma_start(wi0_sb[:], p_wi0[:])
            nc.sync.dma_start(em_sb[:, 0:H], p_embed[0:128, :])
            nc.sync.dma_start(em_sb[:, H : 2 * H], p_embed[128:V, :])
            for k in range(KT):
                for vh in range(2):
                    pt = pps.tile([128, 128], DTR, tag="ptr")
                    nc.tensor.transpose(
                        r(pt[:]),
                        r(em_sb[:, vh * H + k * 128 : vh * H + (k + 1) * 128]),
                        r(ident_sb[:]),
                    )
                    nc.vector.tensor_copy(
                        emt_sb[:, k * V + vh * 128 : k * V + (vh + 1) * 128], pt[:]
                    )
            for m in range(2):
                ps = pps.tile([128, GC], DT, tag="pwie")
                nc.tensor.matmul(
                    ps[:], r(onesr_sb[:]), r(b0_sb[:]), start=True, stop=False
                )
                for k in range(KT):
                    nc.tensor.matmul(
                        ps[:],
                        r(emt_sb[:, k * V + m * 128 : k * V + (m + 1) * 128]),
                        r(wi0_sb[:, k * GC : (k + 1) * GC]),
                        start=False,
                        stop=(k == KT - 1),
                    )
                nc.vector.tensor_copy(wie_sb[:, m * GC : (m + 1) * GC], ps[:])

        hT = stack.enter_context(tc.tile_pool(name="hT", bufs=3))
        cst = stack.enter_context(tc.tile_pool(name="cstate", bufs=3))
        oh = stack.enter_context(tc.tile_pool(name="onehot", bufs=2))
        gp = stack.enter_context(tc.tile_pool(name="gates", bufs=3))
        tp = stack.enter_context(tc.tile_pool(name="tmp", bufs=4))
        dr = stack.enter_context(tc.tile_pool(name="dram", bufs=3, space="DRAM"))
        zp = stack.enter_context(tc.tile_pool(name="zpsum", bufs=2, space="PSUM"))
        tps = stack.enter_context(tc.tile_pool(name="tpsum", bufs=2, space="PSUM"))
        pps2 = stack.enter_context(tc.tile_pool(name="ppsum", bufs=1, space="PSUM"))
        dmp = stack.enter_context(tc.tile_pool(name="dmpsum", bufs=1, space="PSUM"))

        def cell(z, c_prev, tagpfx):
            ga = gp.tile([64, GC], DT, tag=tagpfx + "ga")
            nc.scalar.activation(ga[:], z[:], AF.Tanh)
            sg = gp.tile([64, 384], DT, tag=tagpfx + "sg")
            nc.vector.tensor_scalar(
                sg[:], ga[:, 0:384], 0.5, 0.5, ALU.mult, ALU.add
            )
            ig = tp.tile([64, HC], DT, tag=tagpfx + "ig")
            nc.vector.tensor_tensor(ig[:], sg[:, 0:128], ga[:, 384:512], ALU.mult)
            cf = tp.tile([64, HC], DT, tag=tagpfx + "cf")
            nc.vector.tensor_tensor(cf[:], c_prev[:], sg[:, 128:256], ALU.mult)
            c_new = cst.tile([64, HC], DT, tag=tagpfx + "c")
            nc.vector.tensor_tensor(c_new[:], ig[:], cf[:], ALU.add)
            th = tp.tile([64, HC], DT, tag=tagpfx + "th")
            nc.scalar.activation(th[:], c_new[:], AF.Tanh)
            h = tp.tile([64, HC], DTR, tag=tagpfx + "h")
            nc.vector.tensor_tensor(h[:], sg[:, 256:384], th[:], ALU.mult)
            return h, c_new

        for _ in range(reps):
            h1F_cur = zero_sb      # h1full(w-1) at tick w
            h1F_prev = zero_sb     # h1full(w-2)
            h2F_cur = zero_sb      # h2full(w-3)
            c1 = cst.tile([64, HC], DT, tag="1c")
            c2 = cst.tile([64, HC], DT, tag="2c")
            nc.vector.memset(c1[:], 0.0)
            nc.vector.memset(c2[:], 0.0)

            ohlo = ohhi = None
            for w in range(t_steps + 3):
                do1 = w < t_steps
                do2 = 2 <= w <= t_steps + 1
                dop = 3 <= w <= t_steps + 2
                doAG = w <= t_steps + 1
                if do1:
                    j = w % CH
                    if j == 0:
                        nch = min(CH, t_steps - w)
                        ohlo = oh.tile([128, CH * B], DTR, tag="ohlo")
                        ohhi = oh.tile([128, CH * B], DTR, tag="ohhi")
                        nc.sync.dma_start(
                            ohlo[:, 0 : nch * B], p_oh[0, :, w * B : (w + nch) * B]
                        )
                        nc.sync.dma_start(
                            ohhi[:, 0 : nch * B], p_oh[1, :, w * B : (w + nch) * B]
                        )
                # --- ungated lead-ins (execute inside the AG wait window) ---
                z1 = z2 = pj = None
                if do1:
                    z1 = zp.tile([64, GC], DT, tag="z1")
                    nc.tensor.matmul(
                        z1[:], r(ohlo[:, j * B : (j + 1) * B]), r(wie_sb[:, 0:GC]),
                        start=True, stop=False,
                    )
                    nc.tensor.matmul(
                        z1[:], r(ohhi[:, j * B : (j + 1) * B]),
                        r(wie_sb[:, GC : 2 * GC]),
                        start=False, stop=False,
                    )
                if do2:
                    z2 = zp.tile([64, GC], DT, tag="z2")
                    nc.tensor.matmul(
                        z2[:], r(ones_sb[:]), r(b1_sb[:]), start=True, stop=False
                    )
                    for k in range(KT):
                        nc.tensor.matmul(
                            z2[:],
                            rh(h1F_prev[:, k * B : (k + 1) * B]),
                            rh(wi1_sb[:, k * GC : (k + 1) * GC]),
                            start=False, stop=False,
                        )
                if w > 0:
                    dmy = dmp.tile([1, GC], DT, tag="dmy")
                    for _d in range(N_WARM):
                        nc.tensor.matmul(
                            dmy[:], r(ones_sb[0:1, 0:1]), r(wie_sb[0:1, 0:GC]),
                            start=True, stop=True, skip_group_check=True,
                        )
                # --- gated contractions: z1 chain first (critical) ---
                if do1:
                    for k in range(KT):
                        nc.tensor.matmul(
                            z1[:],
                            rh(h1F_cur[:, k * B : (k + 1) * B]),
                            rh(wh0_sb[:, k * GC : (k + 1) * GC]),
                            start=False, stop=(k == KT - 1),
                        )
                if do2:
                    for k in range(KT):
                        nc.tensor.matmul(
                            z2[:],
                            rh(h2F_cur[:, k * B : (k + 1) * B]),
                            rh(wh1_sb[:, k * GC : (k + 1) * GC]),
                            start=False, stop=(k == KT - 1),
                        )
                # --- cell1 -> transpose1 -> agin1 -> AG1 (critical cycle) ---
                h1F_nxt = h2F_nxt = None
                if do1:
                    h1, c1 = cell(z1, c1, "1")
                    pt1 = tps.tile([128, B], DTR, tag="pt")
                    nc.tensor.transpose(r(pt1[:]), r(h1[:]), r(ident_sb[0:64, 0:64]))
                    ob1 = tp.tile([128, B], DH, tag="ob1")
                    nc.vector.tensor_copy(ob1[:], pt1[:].bitcast(DT) if half else pt1[:])
                    if local:
                        h1F_nxt = hT.tile([128, NC * B], DH, tag="h1F")
                        nc.gpsimd.dma_start(
                            h1F_nxt[:],
                            c_zeroh[:] if half else c_zero[:].bitcast(DTR))
                        nc.vector.tensor_copy(h1F_nxt[:, 0:B], ob1[:])
                    else:
                        agin1 = dr.tile([128, B], DH, tag="agin1")
                        agout1 = dr.tile([NC * 128, B], DH, tag="agout1")
                        nc.sync.dma_start(agin1[:], ob1[:])
                        nc.gpsimd.collective_compute(
                            "AllGather",
                            ALU.bypass,
                            replica_groups=[list(range(NC))],
                            ins=[agin1[:].opt()],
                            outs=[agout1[:].opt()],
                        )
                        h1F_nxt = hT.tile([128, NC * B], DH, tag="h1F")
                        src = agout1[:].rearrange("(s p) j -> p s j", s=NC, p=128)
                        dst = h1F_nxt[:].rearrange("p (s j) -> p s j", s=NC, j=B)
                        nc.sync.dma_start(dst, src)
                # proj in the wait window
                if dop:
                    pj = pps2.tile([64, V], DT, tag="pj")
                    for k in range(KT):
                        nc.tensor.matmul(
                            pj[:],
                            rh(h2F_cur[:, k * B : (k + 1) * B]),
                            rh(wproj_sb[:, k * V : (k + 1) * V]),
                            start=(k == 0),
                            stop=(k == KT - 1),
                        )
                # --- cell2 -> transpose2 -> agin2 -> AG2 (one tick of slack) ---
                if do2:
                    h2, c2 = cell(z2, c2, "2")
                    pt2 = tps.tile([128, B], DTR, tag="pt")
                    nc.tensor.transpose(r(pt2[:]), r(h2[:]), r(ident_sb[0:64, 0:64]))
                    ob2 = tp.tile([128, B], DH, tag="ob2")
                    nc.vector.tensor_copy(ob2[:], pt2[:].bitcast(DT) if half else pt2[:])
                    if local:
                        h2F_nxt = hT.tile([128, NC * B], DH, tag="h2F")
                        nc.gpsimd.dma_start(
                            h2F_nxt[:],
                            c_zeroh[:] if half else c_zero[:].bitcast(DTR))
                        nc.vector.tensor_copy(h2F_nxt[:, 0:B], ob2[:])
                    else:
                        agin2 = dr.tile([128, B], DH, tag="agin2")
                        agout2 = dr.tile([NC * 128, B], DH, tag="agout2")
                        nc.scalar.dma_start(agin2[:], ob2[:])
                        nc.gpsimd.collective_compute(
                            "AllGather",
                            ALU.bypass,
                            replica_groups=[list(range(NC))],
                            ins=[agin2[:].opt()],
                            outs=[agout2[:].opt()],
                        )
                        h2F_nxt = hT.tile([128, NC * B], DH, tag="h2F")
                        src = agout2[:].rearrange("(s p) j -> p s j", s=NC, p=128)
                        dst = h2F_nxt[:].rearrange("p (s j) -> p s j", s=NC, j=B)
                        nc.scalar.dma_start(dst, src)
                if dop:
                    lo = tp.tile([64, V], DT, tag="lo")
                    nc.vector.tensor_copy(lo[:], pj[:])
                    nc.sync.dma_start(p_out[:, w - 3, :], lo[:])
                if h1F_nxt is not None:
                    h1F_prev = h1F_cur
                    h1F_cur = h1F_nxt
                else:
                    h1F_prev = h1F_cur
                if h2F_nxt is not None:
                    h2F_cur = h2F_nxt

    nc.compile()
    return nc


def prep_inputs3(idx, embed, Wi, Wh, b, Wproj, t_steps=T, comm="agc"):
    """Host-side sharding. For rdma3 the k-tile order of the h-contraction
    weights is permuted per core: slot d holds chunk (core ^ d)."""
    order = [0, 1, 3, 2]  # i, f, o, g  (sigmoid block contiguous)
    sc = np.concatenate([np.full(384, 0.5, np.float32), np.ones(128, np.float32)])
    rdma = comm.startswith("rdma3")

    def mov(a, perm):
        t = a.reshape(KT, 128, -1)[perm]
        return np.ascontiguousarray(t.transpose(1, 0, 2).reshape(128, -1))

    idxf = idx[:, :t_steps].T.reshape(-1)
    onehot = (
        (idxf[None, :] == np.arange(V, dtype=idxf.dtype)[:, None])
        .astype(np.float32)
        .reshape(2, 128, t_steps * B)
    )
    iperm = np.arange(KT)
    wdt = np.float16 if comm.startswith("agc4h") else np.float32
    in_maps = []
    for c in range(NC):
        perm = np.array([c ^ d for d in range(NC)]) if rdma else iperm
        cols = np.concatenate(
            [np.arange(q * H + c * HC, q * H + (c + 1) * HC) for q in order]
        )
        m = {
            "embed": np.ascontiguousarray(embed),
            "wi0": mov(Wi[0][:, cols] * sc, iperm),
            "wh0": mov(Wh[0][:, cols] * sc, perm).astype(wdt),
            "wi1": mov(Wi[1][:, cols] * sc, perm).astype(wdt),
            "wh1": mov(Wh[1][:, cols] * sc, perm).astype(wdt),
            "b0": np.ascontiguousarray(b[0][cols] * sc).reshape(1, GC),
            "b1": np.ascontiguousarray(b[1][cols] * sc).reshape(1, GC),
            "wproj": mov(Wproj, perm).astype(wdt),
            "onehot": onehot,
        }
        in_maps.append({k: v.astype(v.dtype, copy=False) for k, v in m.items()})
    return in_maps


_NC_CACHE = {}


def _get_nc(t_steps, reps, comm):
    key = (t_steps, reps, comm)
    if key not in _NC_CACHE:
        _NC_CACHE[key] = build_nc4(t_steps, reps, comm)
    return _NC_CACHE[key]


def run(idx, embed, Wi, Wh, b, Wproj, t_steps=T, reps=1, comm=COMM):
    nc = _get_nc(t_steps, reps, comm)
    in_maps = prep_inputs3(idx, embed, Wi, Wh, b, Wproj, t_steps, comm)
    res = run_bass_kernel_spmd(nc, in_maps, core_ids=list(range(NC)))
    return res.results[0]["out"]


def kernel(idx, embed, Wi, Wh, b, Wproj):
    out = run(
        np.asarray(idx), np.asarray(embed), np.asarray(Wi), np.asarray(Wh),
        np.asarray(b), np.asarray(Wproj),
    )
    return np.asarray(out, dtype=np.float32)


# revision 4
# speedup vs baseline: 1.0881x; 1.0881x over previous
"""Trainium2 Bass kernel for a 2-layer char-LSTM (B=64, T=512, H=1024, V=256).

Strategy (8-way tensor-parallel over the 4H gate dim, one NeuronCore chip):
- Core c owns gate columns [i_c|f_c|o_c|g_c] (128 each, gate-reordered so the
  sigmoid block is contiguous) and hidden chunk c per layer; sigmoid runs via
  the tanh identity with i/f/o weight columns prescaled by 0.5 (the HW sigmoid
  table is too inaccurate).
- The embedding + layer-0 input projection fold into Wie = embed @ Wi0_c + b0
  computed in an on-device prologue; per step the x@Wi0 term is two one-hot
  matmuls accumulated straight into the gate PSUM.
- Per tick, the only serial-critical chain is
    [h1 slots] -> z1 wh0 matmuls -> cell1 -> transpose -> DRAM -> AllGather;
  layer 2 runs two steps behind (z2 at tick w computes h2(w-2)), so its
  gate matmuls, cell, projection and its own AllGather all execute inside
  layer 1's AllGather wait window.  Each layer gathers through its own
  collective; h1 uses the SP HWDGE ring and h2 the Activation ring.
- The gathered chunks load back with one strided rearrange DMA per layer;
  keep-warm dummy matmuls bridge the collective wait so the PE HAM clock
  gate stays at 8/8 (2.4 GHz).
- Direct core-to-core SBUF DMA (remote_dma / hostgen) would cut the ~10us
  per-tick collective cost to ~3us but is non-functional in this
  environment (Q7 SWDGE descriptors crash the exec unit for every routing
  id; REMOTE_DMA_HOSTGEN aborts walrus codegen).
"""
import sys

sys.path.insert(0, "/opt/trn_rl_repo")

import numpy as np
from concourse import bacc, tile, mybir
from concourse.bass_utils import run_bass_kernel_spmd

B, T, H, V, NC = 64, 512, 1024, 256, 8
KT = H // 128
HC = H // NC             # 128 hidden dims per core
GC = 4 * H // NC         # 512 gate cols per core
CH = 16                  # steps per onehot chunk

DT = mybir.dt.float32
DTR = mybir.dt.float32r
AF = mybir.ActivationFunctionType
ALU = mybir.AluOpType

N_WARM = 12  # keep-warm dummy matmuls per tick

COMM = "agc4h"  # default variant (fp16 h gathers)


def r(ap):
    return ap.bitcast(DTR)


def build_nc4(t_steps=T, reps=1, comm="agc4"):
    """wave4: double-lagged layer 2 so only [slots -> z1 -> cell1 -> agin ->
    AllGather] sits on the per-tick critical cycle; z2 (computing h2(w-2)),
    proj (logits(w-3)) and the onehot/bias/wi1 matmuls all execute inside
    the AllGather wait window.  Slot loads alternate between the SP and
    Activation HWDGE rings to avoid single-ring FIFO serialization."""
    nc = bacc.Bacc(None, target_bir_lowering=False, num_devices=NC)

    WDT = mybir.dt.float16 if comm.startswith("agc4h") else DTR
    p_embed = nc.declare_dram_parameter("embed", [V, H], DTR, isOutput=False)
    p_wi0 = nc.declare_dram_parameter("wi0", [128, KT * GC], DTR, isOutput=False)
    p_wh0 = nc.declare_dram_parameter("wh0", [128, KT * GC], WDT, isOutput=False)
    p_wi1 = nc.declare_dram_parameter("wi1", [128, KT * GC], WDT, isOutput=False)
    p_wh1 = nc.declare_dram_parameter("wh1", [128, KT * GC], WDT, isOutput=False)
    p_wproj = nc.declare_dram_parameter("wproj", [128, KT * V], WDT, isOutput=False)
    p_b0 = nc.declare_dram_parameter("b0", [1, GC], DTR, isOutput=False)
    p_b1 = nc.declare_dram_parameter("b1", [1, GC], DTR, isOutput=False)
    p_oh = nc.declare_dram_parameter(
        "onehot", [2, 128, t_steps * B], DTR, isOutput=False
    )
    p_out = nc.declare_dram_parameter("out", [B, t_steps, V], DT, isOutput=True)

    c_ident = nc.inline_tensor(np.eye(128, dtype=np.float32), name="ident")
    c_ones = nc.inline_tensor(np.ones((1, B), dtype=np.float32), name="ones")
    c_onesr = nc.inline_tensor(np.ones((1, 128), dtype=np.float32), name="onesr")
    c_zero = nc.inline_tensor(
        np.zeros((128, NC * B), dtype=np.float32), name="zeros"
    )
    c_zeroh = nc.inline_tensor(
        np.zeros((128, NC * B), dtype=np.float16), name="zerosh"
    )
    c_onesh = nc.inline_tensor(np.ones((1, B), dtype=np.float16), name="onesh")

    local = comm.endswith("_local")
    half = comm.startswith("agc4h")
    DH = mybir.dt.float16 if half else DTR

    def rh(ap):
        return ap if half else ap.bitcast(DTR)

    from contextlib import ExitStack

    with tile.TileContext(nc) as tc, ExitStack() as stack:
        wp = stack.enter_context(tc.tile_pool(name="weights", bufs=1))
        wh0_sb = wp.tile([128, KT * GC], DH, tag="wh0")
        wi1_sb = wp.tile([128, KT * GC], DH, tag="wi1")
        wh1_sb = wp.tile([128, KT * GC], DH, tag="wh1")
        wproj_sb = wp.tile([128, KT * V], DH, tag="wproj")
        wie_sb = wp.tile([128, 2 * GC], DTR, tag="wie")
        b1_sb = wp.tile([1, GC], DTR, tag="b1")
        b0_sb = wp.tile([1, GC], DTR, tag="b0")
        ident_sb = wp.tile([128, 128], DTR, tag="ident")
        ones_sb = wp.tile([1, B], DTR, tag="ones")
        onesr_sb = wp.tile([1, 128], DTR, tag="onesr")
        zero_sb = wp.tile([128, NC * B], DH, tag="zero")

        nc.sync.dma_start(wh0_sb[:], p_wh0[:])
        nc.sync.dma_start(wi1_sb[:], p_wi1[:])
        nc.sync.dma_start(wh1_sb[:], p_wh1[:])
        nc.sync.dma_start(wproj_sb[:], p_wproj[:])
        nc.sync.dma_start(b1_sb[:], p_b1[:])
        nc.sync.dma_start(b0_sb[:], p_b0[:])
        nc.gpsimd.dma_start(ident_sb[:], c_ident[:])
        nc.gpsimd.dma_start(ones_sb[:], c_ones[:])
        nc.gpsimd.dma_start(onesr_sb[:], c_onesr[:])
        if half:
            nc.gpsimd.dma_start(zero_sb[:], c_zeroh[:])
        else:
            nc.gpsimd.dma_start(zero_sb[:], c_zero[:].bitcast(DTR))

        with (
            tc.tile_pool(name="prolog", bufs=1) as pp,
            tc.tile_pool(name="prolog_ps", bufs=2, space="PSUM") as pps,
        ):
            wi0_sb = pp.tile([128, KT * GC], DTR, tag="wi0")
            em_sb = pp.tile([128, 2 * H], DTR, tag="em")
            emt_sb = pp.tile([128, KT * V], DTR, tag="emt")
            nc.sync.dma_start(wi0_sb[:], p_wi0[:])
            nc.sync.dma_start(em_sb[:, 0:H], p_embed[0:128, :])
            nc.sync.dma_start(em_sb[:, H : 2 * H], p_embed[128:V, :])
            for k in range(KT):
                for vh in range(2):
                    pt = pps.tile([128, 128], DTR, tag="ptr")
                    nc.tensor.transpose(
                        r(pt[:]),
                        r(em_sb[:, vh * H + k * 128 : vh * H + (k + 1) * 128]),
                        r(ident_sb[:]),
                    )
                    nc.vector.tensor_copy(
                        emt_sb[:, k * V + vh * 128 : k * V + (vh + 1) * 128], pt[:]
                    )
            for m in range(2):
                ps = pps.tile([128, GC], DT, tag="pwie")
                nc.tensor.matmul(
                    ps[:], r(onesr_sb[:]), r(b0_sb[:]), start=True, stop=False
                )
                for k in range(KT):
                    nc.tensor.matmul(
                        ps[:],
                        r(emt_sb[:, k * V + m * 128 : k * V + (m + 1) * 128]),
                        r(wi0_sb[:, k * GC : (k + 1) * GC]),
                        start=False,
                        stop=(k == KT - 1),
                    )
                nc.vector.tensor_copy(wie_sb[:, m * GC : (m + 1) * GC], ps[:])

        hT = stack.enter_context(tc.tile_pool(name="hT", bufs=3))
        cst = stack.enter_context(tc.tile_pool(name="cstate", bufs=3))
        oh = stack.enter_context(tc.tile_pool(name="onehot", bufs=2))
        gp = stack.enter_context(tc.tile_pool(name="gates", bufs=3))
        tp = stack.enter_context(tc.tile_pool(name="tmp", bufs=4))
        dr = stack.enter_context(tc.tile_pool(name="dram", bufs=3, space="DRAM"))
        zp = stack.enter_context(tc.tile_pool(name="zpsum", bufs=2, space="PSUM"))
        tps = stack.enter_context(tc.tile_pool(name="tpsum", bufs=2, space="PSUM"))
        pps2 = stack.enter_context(tc.tile_pool(name="ppsum", bufs=1, space="PSUM"))
        dmp = stack.enter_context(tc.tile_pool(name="dmpsum", bufs=1, space="PSUM"))

        def cell(z, c_prev, tagpfx):
            ga = gp.tile([64, GC], DT, tag=tagpfx + "ga")
            nc.scalar.activation(ga[:], z[:], AF.Tanh)
            sg = gp.tile([64, 384], DT, tag=tagpfx + "sg")
            nc.vector.tensor_scalar(
                sg[:], ga[:, 0:384], 0.5, 0.5, ALU.mult, ALU.add
            )
            ig = tp.tile([64, HC], DT, tag=tagpfx + "ig")
            nc.vector.tensor_tensor(ig[:], sg[:, 0:128], ga[:, 384:512], ALU.mult)
            cf = tp.tile([64, HC], DT, tag=tagpfx + "cf")
            nc.vector.tensor_tensor(cf[:], c_prev[:], sg[:, 128:256], ALU.mult)
            c_new = cst.tile([64, HC], DT, tag=tagpfx + "c")
            nc.vector.tensor_tensor(c_new[:], ig[:], cf[:], ALU.add)
            th = tp.tile([64, HC], DT, tag=tagpfx + "th")
            nc.scalar.activation(th[:], c_new[:], AF.Tanh)
            h = tp.tile([64, HC], DTR, tag=tagpfx + "h")
            nc.vector.tensor_tensor(h[:], sg[:, 256:384], th[:], ALU.mult)
            return h, c_new

        for _ in range(reps):
            h1F_cur = zero_sb      # h1full(w-1) at tick w
            h1F_prev = zero_sb     # h1full(w-2)
            h2F_cur = zero_sb      # h2full(w-3)
            c1 = cst.tile([64, HC], DT, tag="1c")
            c2 = cst.tile([64, HC], DT, tag="2c")
            nc.vector.memset(c1[:], 0.0)
            nc.vector.memset(c2[:], 0.0)

            ohlo = ohhi = None
            for w in range(t_steps + 3):
                do1 = w < t_steps
                do2 = 2 <= w <= t_steps + 1
                dop = 3 <= w <= t_steps + 2
                doAG = w <= t_steps + 1
                if do1:
                    j = w % CH
                    if j == 0:
                        nch = min(CH, t_steps - w)
                        ohlo = oh.tile([128, CH * B], DTR, tag="ohlo")
                        ohhi = oh.tile([128, CH * B], DTR, tag="ohhi")
                        nc.sync.dma_start(
                            ohlo[:, 0 : nch * B], p_oh[0, :, w * B : (w + nch) * B]
                        )
                        nc.sync.dma_start(
                            ohhi[:, 0 : nch * B], p_oh[1, :, w * B : (w + nch) * B]
                        )
                # --- ungated lead-ins (execute inside the AG wait window) ---
                z1 = z2 = pj = None
                if do1:
                    z1 = zp.tile([64, GC], DT, tag="z1")
                    nc.tensor.matmul(
                        z1[:], r(ohlo[:, j * B : (j + 1) * B]), r(wie_sb[:, 0:GC]),
                        start=True, stop=False,
                    )
                    nc.tensor.matmul(
                        z1[:], r(ohhi[:, j * B : (j + 1) * B]),
                        r(wie_sb[:, GC : 2 * GC]),
                        start=False, stop=False,
                    )
                if do2:
                    z2 = zp.tile([64, GC], DT, tag="z2")
                    nc.tensor.matmul(
                        z2[:], r(ones_sb[:]), r(b1_sb[:]), start=True, stop=False
                    )
                    for k in range(KT):
                        nc.tensor.matmul(
                            z2[:],
                            rh(h1F_prev[:, k * B : (k + 1) * B]),
                            rh(wi1_sb[:, k * GC : (k + 1) * GC]),
                            start=False, stop=False,
                        )
                if w > 0:
                    dmy = dmp.tile([1, GC], DT, tag="dmy")
                    for _d in range(N_WARM):
                        nc.tensor.matmul(
                            dmy[:], r(ones_sb[0:1, 0:1]), r(wie_sb[0:1, 0:GC]),
                            start=True, stop=True, skip_group_check=True,
                        )
                # --- gated contractions: z1 chain first (critical) ---
                if do1:
                    for k in range(KT):
                        nc.tensor.matmul(
                            z1[:],
                            rh(h1F_cur[:, k * B : (k + 1) * B]),
                            rh(wh0_sb[:, k * GC : (k + 1) * GC]),
                            start=False, stop=(k == KT - 1),
                        )
                if do2:
                    for k in range(KT):
                        nc.tensor.matmul(
                            z2[:],
                            rh(h2F_cur[:, k * B : (k + 1) * B]),
                            rh(wh1_sb[:, k * GC : (k + 1) * GC]),
                            start=False, stop=(k == KT - 1),
                        )
                # --- cell1 -> transpose1 -> agin1 -> AG1 (critical cycle) ---
                h1F_nxt = h2F_nxt = None
                if do1:
                    h1, c1 = cell(z1, c1, "1")
                    pt1 = tps.tile([128, B], DTR, tag="pt")
                    nc.tensor.transpose(r(pt1[:]), r(h1[:]), r(ident_sb[0:64, 0:64]))
                    ob1 = tp.tile([128, B], DH, tag="ob1")
                    nc.vector.tensor_copy(ob1[:], pt1[:].bitcast(DT) if half else pt1[:])
                    if local:
                        h1F_nxt = hT.tile([128, NC * B], DH, tag="h1F")
                        nc.gpsimd.dma_start(
                            h1F_nxt[:],
                            c_zeroh[:] if half else c_zero[:].bitcast(DTR))
                        nc.vector.tensor_copy(h1F_nxt[:, 0:B], ob1[:])
                    else:
                        agin1 = dr.tile([128, B], DH, tag="agin1")
                        agout1 = dr.tile([NC * 128, B], DH, tag="agout1")
                        nc.sync.dma_start(agin1[:], ob1[:])
                        nc.gpsimd.collective_compute(
                            "AllGather",
                            ALU.bypass,
                            replica_groups=[list(range(NC))],
                            ins=[agin1[:].opt()],
                            outs=[agout1[:].opt()],
                        )
                        h1F_nxt = hT.tile([128, NC * B], DH, tag="h1F")
                        hf = NC // 2
                        sa = agout1[0 : hf * 128, :].rearrange(
                            "(s p) j -> p s j", s=hf, p=128)
                        da = h1F_nxt[:, 0 : hf * B].rearrange(
                            "p (s j) -> p s j", s=hf, j=B)
                        nc.sync.dma_start(da, sa)
                        sb_ = agout1[hf * 128 : NC * 128, :].rearrange(
                            "(s p) j -> p s j", s=hf, p=128)
                        db = h1F_nxt[:, hf * B : NC * B].rearrange(
                            "p (s j) -> p s j", s=hf, j=B)
                        nc.scalar.dma_start(db, sb_)
                # proj in the wait window
                if dop:
                    pj = pps2.tile([64, V], DT, tag="pj")
                    for k in range(KT):
                        nc.tensor.matmul(
                            pj[:],
                            rh(h2F_cur[:, k * B : (k + 1) * B]),
                            rh(wproj_sb[:, k * V : (k + 1) * V]),
                            start=(k == 0),
                            stop=(k == KT - 1),
                        )
                # --- cell2 -> transpose2 -> agin2 -> AG2 (one tick of slack) ---
                if do2:
                    h2, c2 = cell(z2, c2, "2")
                    pt2 = tps.tile([128, B], DTR, tag="pt")
                    nc.tensor.transpose(r(pt2[:]), r(h2[:]), r(ident_sb[0:64, 0:64]))
                    ob2 = tp.tile([128, B], DH, tag="ob2")
                    nc.vector.tensor_copy(ob2[:], pt2[:].bitcast(DT) if half else pt2[:])
                    if local:
                        h2F_nxt = hT.tile([128, NC * B], DH, tag="h2F")
                        nc.gpsimd.dma_start(
                            h2F_nxt[:],
                            c_zeroh[:] if half else c_zero[:].bitcast(DTR))
                        nc.vector.tensor_copy(h2F_nxt[:, 0:B], ob2[:])
                    else:
                        agin2 = dr.tile([128, B], DH, tag="agin2")
                        agout2 = dr.tile([NC * 128, B], DH, tag="agout2")
                        nc.scalar.dma_start(agin2[:], ob2[:])
                        nc.gpsimd.collective_compute(
                            "AllGather",
                            ALU.bypass,
                            replica_groups=[list(range(NC))],
                            ins=[agin2[:].opt()],
                            outs=[agout2[:].opt()],
                        )
                        h2F_nxt = hT.tile([128, NC * B], DH, tag="h2F")
                        hf = NC // 2
                        sa = agout2[0 : hf * 128, :].rearrange(
                            "(s p) j -> p s j", s=hf, p=128)
                        da = h2F_nxt[:, 0 : hf * B].rearrange(
                            "p (s j) -> p s j", s=hf, j=B)
                        nc.scalar.dma_start(da, sa)
                        sb_ = agout2[hf * 128 : NC * 128, :].rearrange(
                            "(s p) j -> p s j", s=hf, p=128)
                        db = h2F_nxt[:, hf * B : NC * B].rearrange(
                            "p (s j) -> p s j", s=hf, j=B)
                        nc.sync.dma_start(db, sb_)
                if dop:
                    lo = tp.tile([64, V], DT, tag="lo")
                    nc.vector.tensor_copy(lo[:], pj[:])
                    nc.sync.dma_start(p_out[:, w - 3, :], lo[:])
                if h1F_nxt is not None:
                    h1F_prev = h1F_cur
                    h1F_cur = h1F_nxt
                else:
                    h1F_prev = h1F_cur
                if h2F_nxt is not None:
                    h2F_cur = h2F_nxt

    nc.compile()
    return nc


def prep_inputs3(idx, embed, Wi, Wh, b, Wproj, t_steps=T, comm="agc"):
    """Host-side sharding. For rdma3 the k-tile order of the h-contraction
    weights is permuted per core: slot d holds chunk (core ^ d)."""
    order = [0, 1, 3, 2]  # i, f, o, g  (sigmoid block contiguous)
    sc = np.concatenate([np.full(384, 0.5, np.float32), np.ones(128, np.float32)])
    rdma = comm.startswith("rdma3")

    def mov(a, perm):
        t = a.reshape(KT, 128, -1)[perm]
        return np.ascontiguousarray(t.transpose(1, 0, 2).reshape(128, -1))

    idxf = idx[:, :t_steps].T.reshape(-1)
    onehot = (
        (idxf[None, :] == np.arange(V, dtype=idxf.dtype)[:, None])
        .astype(np.float32)
        .reshape(2, 128, t_steps * B)
    )
    iperm = np.arange(KT)
    wdt = np.float16 if comm.startswith("agc4h") else np.float32
    in_maps = []
    for c in range(NC):
        perm = np.array([c ^ d for d in range(NC)]) if rdma else iperm
        cols = np.concatenate(
            [np.arange(q * H + c * HC, q * H + (c + 1) * HC) for q in order]
        )
        m = {
            "embed": np.ascontiguousarray(embed),
            "wi0": mov(Wi[0][:, cols] * sc, iperm),
            "wh0": mov(Wh[0][:, cols] * sc, perm).astype(wdt),
            "wi1": mov(Wi[1][:, cols] * sc, perm).astype(wdt),
            "wh1": mov(Wh[1][:, cols] * sc, perm).astype(wdt),
            "b0": np.ascontiguousarray(b[0][cols] * sc).reshape(1, GC),
            "b1": np.ascontiguousarray(b[1][cols] * sc).reshape(1, GC),
            "wproj": mov(Wproj, perm).astype(wdt),
            "onehot": onehot,
        }
        in_maps.append({k: v.astype(v.dtype, copy=False) for k, v in m.items()})
    return in_maps


_NC_CACHE = {}


def _get_nc(t_steps, reps, comm):
    key = (t_steps, reps, comm)
    if key not in _NC_CACHE:
        _NC_CACHE[key] = build_nc4(t_steps, reps, comm)
    return _NC_CACHE[key]


def run(idx, embed, Wi, Wh, b, Wproj, t_steps=T, reps=1, comm=COMM):
    nc = _get_nc(t_steps, reps, comm)
    in_maps = prep_inputs3(idx, embed, Wi, Wh, b, Wproj, t_steps, comm)
    res = run_bass_kernel_spmd(nc, in_maps, core_ids=list(range(NC)))
    return res.results[0]["out"]


def kernel(idx, embed, Wi, Wh, b, Wproj):
    out = run(
        np.asarray(idx), np.asarray(embed), np.asarray(Wi), np.asarray(Wh),
        np.asarray(b), np.asarray(Wproj),
    )
    return np.asarray(out, dtype=np.float32)
